# revision 20
# baseline (speedup 1.0000x reference)
"""CGC (Customized Gate Control) MoE layer on 8 Trainium2 NeuronCores.

Strategy: data-parallel over batch. B=4096 is split into 8 shards of 512
rows; every core holds all 8 expert MLPs (weights replicated in its
in_map) and computes the full layer for its shard - no collectives.

All expert/gate matmuls run as fp8e4 DoubleRow with 3-term error
compensation. Each f32 operand A is host-split into A_hi = fp8(s*A) and
A_lo = fp8(s*A - A_hi); the product x@W is computed as

    x_hi@(W_hi + W_lo) + x_lo@W_hi        (lo*lo term dropped)

by packing hi/lo pairs into the two DoubleRow contraction slots:
  instr1: lhsT slots (W_hi_k, W_lo_k)   rhs slots (x_hi_k, x_hi_k) [stride-0]
  instr2: lhsT slots (W_hi_k, W_hi_k+1) rhs slots (x_lo_k, x_lo_k+1)
Full K=1024 costs 12 DoubleRow instructions per [128,256] output chunk
(DoubleRow = 0.5 cycles/output-row): a 1.33x Tensor speedup over exact
fp32r at ~0.3% relative error.

Per-core dataflow (BL=512 local batch):
  - x arrives pre-transposed/quantized from host as [128, 8kt, 2hl, BL]
    fp8 tiles (no PE transposes at all).
  - L1: chunks drain via ACT (relu + per-partition bias + scale) to a
    f32 temp; GpSimd converts to h_hi fp8, DVE computes h_lo fp8; both
    land in the interleaved hT tile consumed by L2.
  - L2: out natural [b, H2]; b2 enters the PSUM group as a rank-1 bf16
    matmul (ones.T @ b2); drains on ACT as relu -> bf16.
  - Gates: logits computed directly in [b-part, K-free] orientation
    (stationary = xT slots, moving = Wg slots, rank-1 bf16 bias append),
    so no transposes; softmax runs off PSUM on DVE/ACT.
  - Gated combine: single-instruction MAC on VectorE in bf16
    (scalar_tensor_tensor, 2x throughput mode for 2-byte dtypes).
  - Expert schedule is software-pipelined (L2 of expert e runs after L1
    of expert e+1) so PE never waits on the L1 drain chain.
Outputs are bf16, upcast to f32 on host.
"""

import numpy as np
import ml_dtypes

import concourse.tile as tile
from concourse import bacc, mybir
from concourse.bass_utils import run_bass_kernel_spmd

N_CORES = 8
B = 4096
BL = B // N_CORES  # 512 rows per core
D = 1024
H1 = 1024
H2 = 512
DOM = 3
NES = 2
NSH = 2
E_SPEC = DOM * NES  # 6
GATE_K = NES + NSH  # 4
TOTAL_E = E_SPEC + NSH  # 8

F32 = mybir.dt.float32
F32R = mybir.dt.float32r
F8 = mybir.dt.float8e4
F8E5 = mybir.dt.float8e5
BF16 = mybir.dt.bfloat16
AX = mybir.AxisListType
AF = mybir.ActivationFunctionType
ALU = mybir.AluOpType
DR = mybir.MatmulPerfMode.DoubleRow

NPF8 = ml_dtypes.float8_e4m3
NPF8E5 = ml_dtypes.float8_e5m2
NPBF = ml_dtypes.bfloat16

NBT = BL // 128  # 4 batch tiles per core
NK = D // 128    # 8 contraction tiles over D (== over H1)
NM = H1 // 128   # 8 output tiles over H1

SX = 16.0    # x scale before fp8
SW = 512.0   # weight scale before fp8
SH = 16.0    # hidden scale before fp8
S1 = SX * SW  # L1 PSUM units per true unit (8192)
S2 = SH * SW  # L2 PSUM units (8192)

NEXP = TOTAL_E  # device expert order: [shared0, shared1, spec0..spec5]


def _build_nc(mm_dt=None):
    from contextlib import ExitStack

    nc = bacc.Bacc("TRN2", target_bir_lowering=False, debug=False)

    # x order: 0 = x_shared, 1..3 = x0..x2
    xil = [
        nc.dram_tensor(f"xil{t}", [128, NK, 2, BL], F8, kind="ExternalInput")
        for t in range(4)
    ]
    w1a = nc.dram_tensor("w1a", [NEXP, NM, 128, NK, 2, 128], F8, kind="ExternalInput")
    w2a = nc.dram_tensor("w2a", [NEXP, 128, NK, 2, H2], F8, kind="ExternalInput")
    b1a = nc.dram_tensor("b1a", [128, NEXP, NM], F32, kind="ExternalInput")
    b2a = nc.dram_tensor("b2a", [1, NEXP, 2, H2], F8E5, kind="ExternalInput")
    wsga = nc.dram_tensor("wsga", [128, NK, 2, TOTAL_E], F8, kind="ExternalInput")
    wga = nc.dram_tensor("wga", [DOM, 128, NK, 2, GATE_K], F8, kind="ExternalInput")
    bsga = nc.dram_tensor("bsga", [1, TOTAL_E], BF16, kind="ExternalInput")
    bga = nc.dram_tensor("bga", [1, DOM, GATE_K], BF16, kind="ExternalInput")
    ys = [
        nc.dram_tensor(n, [BL, H2], BF16, kind="ExternalOutput")
        for n in ("y0", "y1", "y2", "ysh")
    ]

    with tile.TileContext(nc) as tc, ExitStack() as ctx:
        p_const = ctx.enter_context(tc.tile_pool(name="const", bufs=1))
        p_x = ctx.enter_context(tc.tile_pool(name="x", bufs=3))
        p_w1 = ctx.enter_context(tc.tile_pool(name="w1", bufs=2))
        p_w2 = ctx.enter_context(tc.tile_pool(name="w2", bufs=2))
        p_wg = ctx.enter_context(tc.tile_pool(name="wg", bufs=1))
        p_h = ctx.enter_context(tc.tile_pool(name="hT", bufs=3))
        p_hf = ctx.enter_context(tc.tile_pool(name="hf", bufs=4))
        p_oe = ctx.enter_context(tc.tile_pool(name="oe", bufs=2))
        p_osh = ctx.enter_context(tc.tile_pool(name="osh", bufs=1))
        p_acc = ctx.enter_context(tc.tile_pool(name="acc", bufs=1))
        p_bias = ctx.enter_context(tc.tile_pool(name="bias", bufs=1))
        p_gw = ctx.enter_context(tc.tile_pool(name="gw", bufs=1))
        p_sm = ctx.enter_context(tc.tile_pool(name="sm", bufs=3))
        p_tmp = ctx.enter_context(tc.tile_pool(name="tmp", bufs=4))
        # L1/L2 psum tiles span TWO banks: the two 256-col chunks of an
        # output tile live at cols 0-255 and 512-767 (separate 2KB zero
        # regions), so both groups accumulate concurrently and ONE strided
        # ACT op drains both.
        # 4 double-tiles = all 8 banks; gates and warm-up share the L1 pool.
        ps_l1 = ctx.enter_context(tc.tile_pool(name="psl1", bufs=3, space="PSUM"))
        ps_l2 = ctx.enter_context(tc.tile_pool(name="psl2", bufs=3, space="PSUM"))
        ps_g = ctx.enter_context(tc.tile_pool(name="psg", bufs=2, space="PSUM"))

        warm8 = p_const.tile([128, 2, 256], F8)
        nc.gpsimd.memset(warm8, 0.0)
        onesb = p_const.tile([1, 128], BF16)
        nc.gpsimd.memset(onesb, 1.0)
        ones5 = p_const.tile([1, 2, 128], F8E5)
        nc.gpsimd.memset(ones5, 1.0)

        # PE warm-up: ~4.5us of dummy DoubleRow matmuls while the head DMAs
        # (x_shared + first W1 slabs) land, so the p-state ramp finishes
        # before real work and PE never goes idle at the start.
        for _ in range(40):
            pw = ps_g.tile([128, 512], F32, tag="pg", name="pw")
            nc.tensor.matmul(
                pw[:, :256], lhsT=warm8[:, :, :128], rhs=warm8,
                start=True, stop=True, perf_mode=DR,
            )

        def load_x(t):
            xt = p_x.tile([128, NK, 2, BL], F8, tag="x")
            nc.sync.dma_start(out=xt, in_=xil[t][:])
            return xt

        def load_w1(e, parts=2):
            w1t = p_w1.tile([128, NM, NK, 2, 128], F8, tag="w1")
            src = w1a[e].rearrange("m p k two j -> p m k two j")
            step = NM // parts
            for q in range(parts):
                nc.sync.dma_start(
                    out=w1t[:, q * step : (q + 1) * step],
                    in_=src[:, q * step : (q + 1) * step],
                )
            return w1t

        def load_w2(e):
            w2t = p_w2.tile([128, NK, 2, H2], F8, tag="w2")
            nc.sync.dma_start(out=w2t, in_=w2a[e])
            return w2t

        def gate(xt, wgt, bg_row, K, tag):
            """softmax(x @ Wg + bg) -> gw tile [128, NBT, K], b on partitions.

            The bias rank-1 matmul comes LAST in the PSUM group so the
            bias DMA is not on the critical head path.
            """
            gw = p_gw.tile([128, NBT, K], F32, tag=tag)
            for bt in range(NBT):
                b0 = bt * 128
                pg = ps_g.tile([128, 512], F32, tag="pg", name="pg")
                for kt in range(NK):
                    nc.tensor.matmul(
                        pg[:, :K],
                        lhsT=xt[:, kt, :, b0 : b0 + 128],
                        rhs=wgt[:, kt, 0:1, :].broadcast_to([128, 2, K]),
                        start=(kt == 0), stop=False, perf_mode=DR,
                    )
                for kp in range(NK // 2):
                    nc.tensor.matmul(
                        pg[:, :K],
                        lhsT=xt[:, 2 * kp : 2 * kp + 2, 0, b0 : b0 + 128],
                        rhs=wgt[:, 2 * kp : 2 * kp + 2, 1, :],
                        start=False, stop=False, perf_mode=DR,
                    )
                nc.tensor.matmul(
                    pg[:, :K], lhsT=onesb, rhs=bg_row, start=False, stop=True,
                )
                nm = p_sm.tile([128, 1], F32, tag="nm")
                nc.vector.reduce_max(out=nm, in_=pg[:, :K], axis=AX.X, negate=True)
                nms = p_sm.tile([128, 1], F32, tag="nms")
                nc.vector.tensor_scalar_mul(nms, nm, 1.0 / S1)
                esb = p_sm.tile([128, K], F32, tag=f"esb{K}")
                nc.scalar.activation(
                    out=esb, in_=pg[:, :K], func=AF.Exp, bias=nms, scale=1.0 / S1
                )
                ssb = p_sm.tile([128, 1], F32, tag="ssb")
                nc.vector.reduce_sum(out=ssb, in_=esb, axis=AX.X)
                rsb = p_sm.tile([128, 1], F32, tag="rsb")
                nc.vector.reciprocal(out=rsb, in_=ssb)
                nc.vector.tensor_scalar_mul(gw[:, bt, :], esb, rsb)
            return gw

        def expert_l1(e, xt, w1t):
            """L1: returns interleaved hT fp8 tile [128, NK, 2, BL]."""
            hT = p_h.tile([128, NK, 2, BL], F8, tag="hT")
            for mt in range(NM):
                hf = p_hf.tile([128, BL], F32, tag="hf")
                for cb in range(2):
                    c0 = cb * 256
                    pt = ps_l1.tile([128, 512], F32, tag="l1")
                    for kt in range(NK):
                        nc.tensor.matmul(
                            pt[:, :256],
                            lhsT=w1t[:, mt, kt, :, :],
                            rhs=xt[:, kt, 0:1, c0 : c0 + 256].broadcast_to(
                                [128, 2, 256]
                            ),
                            start=(kt == 0), stop=False, perf_mode=DR,
                        )
                    for kp in range(NK // 2):
                        nc.tensor.matmul(
                            pt[:, :256],
                            lhsT=w1t[:, mt, 2 * kp : 2 * kp + 2, 0, :],
                            rhs=xt[:, 2 * kp : 2 * kp + 2, 1, c0 : c0 + 256],
                            start=False, stop=(kp == NK // 2 - 1), perf_mode=DR,
                        )
                    nc.scalar.activation(
                        out=hf[:, c0 : c0 + 256], in_=pt[:, :256],
                        func=AF.Relu, bias=b1t[:, e, mt : mt + 1], scale=SH / S1,
                    )
                nc.gpsimd.tensor_copy(out=hT[:, mt, 0, :], in_=hf)
                nc.vector.tensor_tensor(
                    hT[:, mt, 1, :], hf, hT[:, mt, 0, :], ALU.subtract
                )
            return hT

        def expert_l2(e, hT, w2t, out_pool, tag):
            """L2: oe tile [128, NBT, H2] bf16 = relu(h @ W2 + b2)."""
            oe = out_pool.tile([128, NBT, H2], BF16, tag=tag)
            for bt in range(NBT):
                b0 = bt * 128
                for cb in range(2):
                    c0 = cb * 256
                    pt = ps_l2.tile([128, 512], F32, tag="l2")
                    nc.tensor.matmul(
                        pt[:, :256],
                        lhsT=ones5, rhs=b2t[0:1, e, :, c0 : c0 + 256],
                        start=True, stop=False, perf_mode=DR,
                    )
                    for kt in range(NK):
                        nc.tensor.matmul(
                            pt[:, :256],
                            lhsT=hT[:, kt, 0:1, b0 : b0 + 128].broadcast_to(
                                [128, 2, 128]
                            ),
                            rhs=w2t[:, kt, :, c0 : c0 + 256],
                            start=False, stop=False, perf_mode=DR,
                        )
                    for kp in range(NK // 2):
                        nc.tensor.matmul(
                            pt[:, :256],
                            lhsT=hT[:, 2 * kp : 2 * kp + 2, 1, b0 : b0 + 128],
                            rhs=w2t[:, 2 * kp : 2 * kp + 2, 0, c0 : c0 + 256],
                            start=False, stop=(kp == NK // 2 - 1), perf_mode=DR,
                        )
                    nc.scalar.activation(
                        out=oe[:, bt, c0 : c0 + 256], in_=pt[:, :256],
                        func=AF.Relu, scale=1.0 / S2,
                    )
            return oe

        accs = [None] * 4

        def accumulate(acc_idx, oe, gw, col, first):
            # mul (4x DVE mode) + add (2x) beats fused scalar_tensor_tensor
            # (no fast mode) under the cost model: 442ns vs 594ns per tile.
            acc = accs[acc_idx]
            for bt in range(NBT):
                if first:
                    nc.vector.tensor_scalar_mul(
                        acc[:, bt, :], oe[:, bt, :], gw[:, bt, col : col + 1]
                    )
                else:
                    tmp = p_tmp.tile([128, H2], BF16, tag="tmp", name="tmp")
                    nc.vector.tensor_scalar_mul(
                        tmp, oe[:, bt, :], gw[:, bt, col : col + 1]
                    )
                    nc.vector.tensor_tensor(
                        acc[:, bt, :], acc[:, bt, :], tmp, ALU.add
                    )

        def accumulate2(oe, gw_a, acc_a, col_a, gw_b, acc_b, col_b):
            """Per-bt interleaved double accumulate (shortens the tail)."""
            for bt in range(NBT):
                for gw, ai, col in ((gw_a, acc_a, col_a), (gw_b, acc_b, col_b)):
                    acc = accs[ai]
                    tmp = p_tmp.tile([128, H2], BF16, tag="tmp", name="tmp")
                    nc.vector.tensor_scalar_mul(
                        tmp, oe[:, bt, :], gw[:, bt, col : col + 1]
                    )
                    nc.vector.tensor_tensor(
                        acc[:, bt, :], acc[:, bt, :], tmp, ALU.add
                    )

        def store(acc_idx, y_dram):
            yr = y_dram[:].rearrange("(bt p) o -> bt p o", p=128)
            for bt in range(NBT):
                nc.sync.dma_start(out=yr[bt], in_=accs[acc_idx][:, bt, :])

        # ---- software-pipelined schedule ----
        # Head DMA order: x_shared + shared-gate weights + first W1 slabs
        # first; everything else after.
        xt_sh = load_x(0)
        w1t = load_w1(0, parts=4)
        b1t = p_bias.tile([128, NEXP, NM], F32, tag="b1")
        nc.sync.dma_start(out=b1t, in_=b1a[:])
        wsgt = p_wg.tile([128, NK, 2, TOTAL_E], F8, tag="wsg")
        nc.sync.dma_start(out=wsgt, in_=wsga[:])
        bsgt = p_bias.tile([1, TOTAL_E], BF16, tag="bsg")
        nc.sync.dma_start(out=bsgt, in_=bsga[:])

        # shared expert 0 (the shared gate runs after it: its result is
        # not needed until the first combine, and putting it later keeps
        # its weight/bias DMAs off the critical head path)
        w1t_n = load_w1(1)
        w2t0 = load_w2(0)
        hT0 = expert_l1(0, xt_sh, w1t)
        gws = gate(xt_sh, wsgt, bsgt, TOTAL_E, tag="gws")
        # shared expert 1
        w1t = w1t_n
        b2t = p_bias.tile([1, NEXP, 2, H2], F8E5, tag="b2")
        nc.sync.dma_start(out=b2t, in_=b2a[:])
        w2t1 = load_w2(1)
        xt0 = load_x(1)
        bgt = p_bias.tile([1, DOM, GATE_K], BF16, tag="bg")
        nc.sync.dma_start(out=bgt, in_=bga[:])
        wgts = []
        for d in range(DOM):
            wgt = p_wg.tile([128, NK, 2, GATE_K], F8, tag=f"wg{d}")
            nc.sync.dma_start(out=wgt, in_=wga[d])
            wgts.append(wgt)
        hT1 = expert_l1(1, xt_sh, w1t)
        osh0 = expert_l2(0, hT0, w2t0, p_osh, tag="osh0")
        gw0 = gate(xt0, wgts[0], bgt[0:1, 0, :], GATE_K, tag="gw0")

        for i in range(4):
            accs[i] = p_acc.tile(
                [128, NBT, H2], BF16, tag=f"acc{i}", name=f"acc{i}"
            )

        # spec e0 (device 2)
        w1t = load_w1(2)
        w2t2 = load_w2(2)
        hT2 = expert_l1(2, xt0, w1t)
        osh1 = expert_l2(1, hT1, w2t1, p_osh, tag="osh1")
        accumulate(3, osh0, gws, E_SPEC + 0, first=True)
        accumulate(3, osh1, gws, E_SPEC + 1, first=False)
        accumulate(0, osh0, gw0, NES + 0, first=True)
        accumulate(0, osh1, gw0, NES + 1, first=False)

        # spec e1 (device 3)
        w1t = load_w1(3)
        w2t3 = load_w2(3)
        xt1 = load_x(2)
        hT3 = expert_l1(3, xt0, w1t)
        oe = expert_l2(2, hT2, w2t2, p_oe, tag="oe")
        accumulate2(oe, gw0, 0, 0, gws, 3, 0)
        gw1 = gate(xt1, wgts[1], bgt[0:1, 1, :], GATE_K, tag="gw1")

        # spec e2 (device 4)
        w1t = load_w1(4)
        w2t4 = load_w2(4)
        hT4 = expert_l1(4, xt1, w1t)
        oe = expert_l2(3, hT3, w2t3, p_oe, tag="oe")
        accumulate2(oe, gw0, 0, 1, gws, 3, 1)
        store(0, ys[0])
        accumulate(1, osh0, gw1, NES + 0, first=True)
        accumulate(1, osh1, gw1, NES + 1, first=False)

        # spec e3 (device 5)
        w1t = load_w1(5)
        w2t5 = load_w2(5)
        xt2 = load_x(3)
        hT5 = expert_l1(5, xt1, w1t)
        oe = expert_l2(4, hT4, w2t4, p_oe, tag="oe")
        accumulate2(oe, gw1, 1, 0, gws, 3, 2)
        gw2 = gate(xt2, wgts[2], bgt[0:1, 2, :], GATE_K, tag="gw2")

        # spec e4 (device 6)
        w1t = load_w1(6)
        w2t6 = load_w2(6)
        hT6 = expert_l1(6, xt2, w1t)
        oe = expert_l2(5, hT5, w2t5, p_oe, tag="oe")
        accumulate2(oe, gw1, 1, 1, gws, 3, 3)
        store(1, ys[1])
        accumulate(2, osh0, gw2, NES + 0, first=True)
        accumulate(2, osh1, gw2, NES + 1, first=False)

        # spec e5 (device 7)
        w1t = load_w1(7)
        w2t7 = load_w2(7)
        hT7 = expert_l1(7, xt2, w1t)
        oe = expert_l2(6, hT6, w2t6, p_oe, tag="oe")
        accumulate2(oe, gw2, 2, 0, gws, 3, 4)

        # tail: per-bt interleaved final combines + stores
        oe = expert_l2(7, hT7, w2t7, p_oe, tag="oe")
        yr2 = ys[2][:].rearrange("(bt p) o -> bt p o", p=128)
        yr3 = ys[3][:].rearrange("(bt p) o -> bt p o", p=128)
        for bt in range(NBT):
            for gw, ai, col, yr in (
                (gw2, 2, 1, yr2),
                (gws, 3, 5, yr3),
            ):
                acc = accs[ai]
                tmp = p_tmp.tile([128, H2], BF16, tag="tmp", name="tmp")
                nc.vector.tensor_scalar_mul(
                    tmp, oe[:, bt, :], gw[:, bt, col : col + 1]
                )
                nc.vector.tensor_tensor(
                    acc[:, bt, :], acc[:, bt, :], tmp, ALU.add
                )
                nc.sync.dma_start(out=yr[bt], in_=acc[:, bt, :])

    nc.compile()
    return nc


_NC_CACHE = {}


def _get_nc(mm_dt=None):
    key = "fp8dr"
    if key not in _NC_CACHE:
        _NC_CACHE[key] = _build_nc()
    return _NC_CACHE[key]


def _hilo(a, s):
    af = np.asarray(a, np.float32) * np.float32(s)
    hi = af.astype(NPF8)
    lo = (af - hi.astype(np.float32)).astype(NPF8)
    return hi, lo


def _prep_inputs(inputs):
    """Quantize/layout all operands for the device (host-side prep)."""
    f = {k: np.asarray(v, np.float32) for k, v in inputs.items()}

    # x tensors: device order [x_shared, x0, x1, x2]
    x_full = [f["x_shared"], f["x0"], f["x1"], f["x2"]]
    x_per_core = []  # [t][core] -> [128, NK, 2, BL] fp8
    for x in x_full:
        hi, lo = _hilo(x, SX)              # [B, D]
        hi = hi.reshape(B, NK, 128)
        lo = lo.reshape(B, NK, 128)
        cores = []
        for c in range(N_CORES):
            sl = slice(c * BL, (c + 1) * BL)
            xa = np.empty((128, NK, 2, BL), NPF8)
            xa[:, :, 0, :] = hi[sl].transpose(2, 1, 0)
            xa[:, :, 1, :] = lo[sl].transpose(2, 1, 0)
            cores.append(xa)
        x_per_core.append(cores)

    # weights: device expert order [shared0, shared1, spec0..spec5]
    W1 = np.concatenate([f["W1h"], f["W1s"]], axis=0)  # [8, D, H1]
    W2 = np.concatenate([f["W2h"], f["W2s"]], axis=0)  # [8, H1, H2]
    b1 = np.concatenate([f["b1h"], f["b1s"]], axis=0)  # [8, H1]
    b2 = np.concatenate([f["b2h"], f["b2s"]], axis=0)  # [8, H2]

    h1i, l1i = _hilo(W1, SW)
    h1i = h1i.reshape(NEXP, NK, 128, NM, 128)
    l1i = l1i.reshape(NEXP, NK, 128, NM, 128)
    w1a = np.empty((NEXP, NM, 128, NK, 2, 128), NPF8)
    w1a[:, :, :, :, 0, :] = h1i.transpose(0, 3, 2, 1, 4)
    w1a[:, :, :, :, 1, :] = l1i.transpose(0, 3, 2, 1, 4)

    h2i, l2i = _hilo(W2, SW)
    h2i = h2i.reshape(NEXP, NK, 128, H2)
    l2i = l2i.reshape(NEXP, NK, 128, H2)
    w2a = np.empty((NEXP, 128, NK, 2, H2), NPF8)
    w2a[:, :, :, 0, :] = h2i.transpose(0, 2, 1, 3)
    w2a[:, :, :, 1, :] = l2i.transpose(0, 2, 1, 3)

    b1a = np.ascontiguousarray(
        (b1 * SH).reshape(NEXP, NM, 128).transpose(2, 0, 1), dtype=np.float32
    )
    b2s = (b2 * S2).astype(np.float32)
    b2hi = b2s.astype(NPF8E5)
    b2lo = (b2s - b2hi.astype(np.float32)).astype(NPF8E5)
    b2a = np.stack([b2hi, b2lo], axis=1).reshape(1, NEXP, 2, H2)

    hsg, lsg = _hilo(f["Wsg"], SW)  # [D, TOTAL_E]
    wsga = np.empty((128, NK, 2, TOTAL_E), NPF8)
    wsga[:, :, 0, :] = hsg.reshape(NK, 128, TOTAL_E).transpose(1, 0, 2)
    wsga[:, :, 1, :] = lsg.reshape(NK, 128, TOTAL_E).transpose(1, 0, 2)

    hg, lg = _hilo(f["Wg"], SW)  # [DOM, D, GATE_K]
    wga = np.empty((DOM, 128, NK, 2, GATE_K), NPF8)
    wga[:, :, :, 0, :] = hg.reshape(DOM, NK, 128, GATE_K).transpose(0, 2, 1, 3)
    wga[:, :, :, 1, :] = lg.reshape(DOM, NK, 128, GATE_K).transpose(0, 2, 1, 3)

    bsga = (f["bsg"] * S1).reshape(1, TOTAL_E).astype(NPBF)
    bga = (f["bg"] * S1).reshape(1, DOM, GATE_K).astype(NPBF)

    shared = {
        "w1a": w1a, "w2a": w2a, "b1a": b1a, "b2a": b2a,
        "wsga": wsga, "wga": wga, "bsga": bsga, "bga": bga,
    }
    in_maps = []
    for c in range(N_CORES):
        m = dict(shared)
        for t in range(4):
            m[f"xil{t}"] = x_per_core[t][c]
        in_maps.append(m)
    return in_maps


def kernel(**inputs):
    return run_kernel(inputs)


def run_kernel(inputs, mm_dt=None, trace=False):
    nc = _get_nc()
    in_maps = _prep_inputs(inputs)
    res = run_bass_kernel_spmd(nc, in_maps, list(range(N_CORES)), trace=trace)
    outs = []
    for name in ("y0", "y1", "y2", "ysh"):
        outs.append(
            np.concatenate(
                [
                    np.asarray(res.results[c][name]).astype(np.float32)
                    for c in range(N_CORES)
                ],
                axis=0,
            )
        )
    out = tuple(outs)
    if trace:
        return out, res
    return out


# revision 21
# speedup vs baseline: 1.0282x; 1.0282x over previous
"""CGC (Customized Gate Control) MoE layer on 8 Trainium2 NeuronCores.

Strategy: data-parallel over batch. B=4096 is split into 8 shards of 512
rows; every core holds all 8 expert MLPs (weights replicated in its
in_map) and computes the full layer for its shard - no collectives.

All expert/gate matmuls run as fp8e4 DoubleRow with 3-term error
compensation. Each f32 operand A is host-split into A_hi = fp8(s*A) and
A_lo = fp8(s*A - A_hi); the product x@W is computed as

    x_hi@(W_hi + W_lo) + x_lo@W_hi        (lo*lo term dropped)

by packing hi/lo pairs into the two DoubleRow contraction slots:
  instr1: lhsT slots (W_hi_k, W_lo_k)   rhs slots (x_hi_k, x_hi_k) [stride-0]
  instr2: lhsT slots (W_hi_k, W_hi_k+1) rhs slots (x_lo_k, x_lo_k+1)
Full K=1024 costs 12 DoubleRow instructions per [128,256] output chunk
(DoubleRow = 0.5 cycles/output-row): a 1.33x Tensor speedup over exact
fp32r at ~0.3% relative error.

Per-core dataflow (BL=512 local batch):
  - x arrives pre-transposed/quantized from host as [128, 8kt, 2hl, BL]
    fp8 tiles (no PE transposes at all).
  - L1: chunks drain via ACT (relu + per-partition bias + scale) to a
    f32 temp; GpSimd converts to h_hi fp8, DVE computes h_lo fp8; both
    land in the interleaved hT tile consumed by L2.
  - L2: out natural [b, H2]; b2 enters the PSUM group as a rank-1 bf16
    matmul (ones.T @ b2); drains on ACT as relu -> bf16.
  - Gates: logits computed directly in [b-part, K-free] orientation
    (stationary = xT slots, moving = Wg slots, rank-1 bf16 bias append),
    so no transposes; softmax runs off PSUM on DVE/ACT.
  - Gated combine: single-instruction MAC on VectorE in bf16
    (scalar_tensor_tensor, 2x throughput mode for 2-byte dtypes).
  - Expert schedule is software-pipelined (L2 of expert e runs after L1
    of expert e+1) so PE never waits on the L1 drain chain.
Outputs are bf16, upcast to f32 on host.
"""

import numpy as np
import ml_dtypes

import concourse.tile as tile
from concourse import bacc, mybir
from concourse.bass_utils import run_bass_kernel_spmd

N_CORES = 8
B = 4096
BL = B // N_CORES  # 512 rows per core
D = 1024
H1 = 1024
H2 = 512
DOM = 3
NES = 2
NSH = 2
E_SPEC = DOM * NES  # 6
GATE_K = NES + NSH  # 4
TOTAL_E = E_SPEC + NSH  # 8

F32 = mybir.dt.float32
F32R = mybir.dt.float32r
F8 = mybir.dt.float8e4
F8E5 = mybir.dt.float8e5
BF16 = mybir.dt.bfloat16
AX = mybir.AxisListType
AF = mybir.ActivationFunctionType
ALU = mybir.AluOpType
DR = mybir.MatmulPerfMode.DoubleRow

NPF8 = ml_dtypes.float8_e4m3
NPF8E5 = ml_dtypes.float8_e5m2
NPBF = ml_dtypes.bfloat16

NBT = BL // 128  # 4 batch tiles per core
NK = D // 128    # 8 contraction tiles over D (== over H1)
NM = H1 // 128   # 8 output tiles over H1

SX = 16.0    # x scale before fp8
SW = 512.0   # weight scale before fp8
SH = 16.0    # hidden scale before fp8
S1 = SX * SW  # L1 PSUM units per true unit (8192)
S2 = SH * SW  # L2 PSUM units (8192)

NEXP = TOTAL_E  # device expert order: [shared0, shared1, spec0..spec5]


def _build_nc(mm_dt=None):
    from contextlib import ExitStack

    nc = bacc.Bacc("TRN2", target_bir_lowering=False, debug=False)

    # x order: 0 = x_shared, 1..3 = x0..x2
    xil = [
        nc.dram_tensor(f"xil{t}", [128, NK, 2, BL], F8, kind="ExternalInput")
        for t in range(4)
    ]
    w1a = nc.dram_tensor("w1a", [NEXP, NM, 128, NK, 2, 128], F8, kind="ExternalInput")
    w2a = nc.dram_tensor("w2a", [NEXP, 128, NK, 2, H2], F8, kind="ExternalInput")
    b1a = nc.dram_tensor("b1a", [128, NEXP, NM], F32, kind="ExternalInput")
    b2a = nc.dram_tensor("b2a", [1, NEXP, 2, H2], F8E5, kind="ExternalInput")
    wsga = nc.dram_tensor("wsga", [128, NK, 2, TOTAL_E], F8, kind="ExternalInput")
    wga = nc.dram_tensor("wga", [DOM, 128, NK, 2, GATE_K], F8, kind="ExternalInput")
    bsga = nc.dram_tensor("bsga", [1, TOTAL_E], BF16, kind="ExternalInput")
    bga = nc.dram_tensor("bga", [1, DOM, GATE_K], BF16, kind="ExternalInput")
    ys = [
        nc.dram_tensor(n, [BL, H2], BF16, kind="ExternalOutput")
        for n in ("y0", "y1", "y2", "ysh")
    ]

    with tile.TileContext(nc) as tc, ExitStack() as ctx:
        p_const = ctx.enter_context(tc.tile_pool(name="const", bufs=1))
        p_x = ctx.enter_context(tc.tile_pool(name="x", bufs=3))
        p_w1 = ctx.enter_context(tc.tile_pool(name="w1", bufs=2))
        p_w2 = ctx.enter_context(tc.tile_pool(name="w2", bufs=2))
        p_wg = ctx.enter_context(tc.tile_pool(name="wg", bufs=1))
        p_h = ctx.enter_context(tc.tile_pool(name="hT", bufs=3))
        p_hf = ctx.enter_context(tc.tile_pool(name="hf", bufs=4))
        p_oe = ctx.enter_context(tc.tile_pool(name="oe", bufs=2))
        p_osh = ctx.enter_context(tc.tile_pool(name="osh", bufs=1))
        p_acc = ctx.enter_context(tc.tile_pool(name="acc", bufs=1))
        p_bias = ctx.enter_context(tc.tile_pool(name="bias", bufs=1))
        p_gw = ctx.enter_context(tc.tile_pool(name="gw", bufs=1))
        p_sm = ctx.enter_context(tc.tile_pool(name="sm", bufs=3))
        p_tmp = ctx.enter_context(tc.tile_pool(name="tmp", bufs=4))
        # L1/L2 psum tiles span TWO banks: the two 256-col chunks of an
        # output tile live at cols 0-255 and 512-767 (separate 2KB zero
        # regions), so both groups accumulate concurrently and ONE strided
        # ACT op drains both.
        # 4 double-tiles = all 8 banks; gates and warm-up share the L1 pool.
        ps_l1 = ctx.enter_context(tc.tile_pool(name="psl1", bufs=3, space="PSUM"))
        ps_l2 = ctx.enter_context(tc.tile_pool(name="psl2", bufs=3, space="PSUM"))
        ps_g = ctx.enter_context(tc.tile_pool(name="psg", bufs=2, space="PSUM"))

        warm8 = p_const.tile([128, 2, 256], F8)
        nc.gpsimd.memset(warm8, 0.0)
        onesb = p_const.tile([1, 128], BF16)
        nc.gpsimd.memset(onesb, 1.0)
        ones5 = p_const.tile([1, 2, 128], F8E5)
        nc.gpsimd.memset(ones5, 1.0)

        # PE warm-up: ~4.5us of dummy DoubleRow matmuls while the head DMAs
        # (x_shared + first W1 slabs) land, so the p-state ramp finishes
        # before real work and PE never goes idle at the start.
        for _ in range(40):
            pw = ps_g.tile([128, 512], F32, tag="pg", name="pw")
            nc.tensor.matmul(
                pw[:, :256], lhsT=warm8[:, :, :128], rhs=warm8,
                start=True, stop=True, perf_mode=DR,
            )

        def load_x(t):
            xt = p_x.tile([128, NK, 2, BL], F8, tag="x")
            nc.sync.dma_start(out=xt, in_=xil[t][:])
            return xt

        def load_w1(e, parts=2):
            w1t = p_w1.tile([128, NM, NK, 2, 128], F8, tag="w1")
            src = w1a[e].rearrange("m p k two j -> p m k two j")
            step = NM // parts
            for q in range(parts):
                nc.sync.dma_start(
                    out=w1t[:, q * step : (q + 1) * step],
                    in_=src[:, q * step : (q + 1) * step],
                )
            return w1t

        def load_w2(e):
            w2t = p_w2.tile([128, NK, 2, H2], F8, tag="w2")
            nc.sync.dma_start(out=w2t, in_=w2a[e])
            return w2t

        def gate(xt, wgt, bg_row, K, tag):
            """softmax(x @ Wg + bg) -> gw tile [128, NBT, K], b on partitions.

            The bias rank-1 matmul comes LAST in the PSUM group so the
            bias DMA is not on the critical head path.
            """
            gw = p_gw.tile([128, NBT, K], F32, tag=tag)
            for bt in range(NBT):
                b0 = bt * 128
                pg = ps_g.tile([128, 512], F32, tag="pg", name="pg")
                for kt in range(NK):
                    nc.tensor.matmul(
                        pg[:, :K],
                        lhsT=xt[:, kt, :, b0 : b0 + 128],
                        rhs=wgt[:, kt, 0:1, :].broadcast_to([128, 2, K]),
                        start=(kt == 0), stop=False, perf_mode=DR,
                    )
                for kp in range(NK // 2):
                    nc.tensor.matmul(
                        pg[:, :K],
                        lhsT=xt[:, 2 * kp : 2 * kp + 2, 0, b0 : b0 + 128],
                        rhs=wgt[:, 2 * kp : 2 * kp + 2, 1, :],
                        start=False, stop=False, perf_mode=DR,
                    )
                nc.tensor.matmul(
                    pg[:, :K], lhsT=onesb, rhs=bg_row, start=False, stop=True,
                )
                nm = p_sm.tile([128, 1], F32, tag="nm")
                nc.vector.reduce_max(out=nm, in_=pg[:, :K], axis=AX.X, negate=True)
                nms = p_sm.tile([128, 1], F32, tag="nms")
                nc.vector.tensor_scalar_mul(nms, nm, 1.0 / S1)
                esb = p_sm.tile([128, K], F32, tag=f"esb{K}")
                nc.scalar.activation(
                    out=esb, in_=pg[:, :K], func=AF.Exp, bias=nms, scale=1.0 / S1
                )
                ssb = p_sm.tile([128, 1], F32, tag="ssb")
                nc.vector.reduce_sum(out=ssb, in_=esb, axis=AX.X)
                rsb = p_sm.tile([128, 1], F32, tag="rsb")
                nc.vector.reciprocal(out=rsb, in_=ssb)
                nc.vector.tensor_scalar_mul(gw[:, bt, :], esb, rsb)
            return gw

        def expert_l1(e, xt, w1t):
            """L1: returns interleaved hT fp8 tile [128, NK, 2, BL]."""
            hT = p_h.tile([128, NK, 2, BL], F8, tag="hT")
            for mt in range(NM):
                hf = p_hf.tile([128, BL], F32, tag="hf")
                for cb in range(2):
                    c0 = cb * 256
                    pt = ps_l1.tile([128, 512], F32, tag="l1")
                    for kt in range(NK):
                        nc.tensor.matmul(
                            pt[:, :256],
                            lhsT=w1t[:, mt, kt, :, :],
                            rhs=xt[:, kt, 0:1, c0 : c0 + 256].broadcast_to(
                                [128, 2, 256]
                            ),
                            start=(kt == 0), stop=False, perf_mode=DR,
                        )
                    for kp in range(NK // 2):
                        nc.tensor.matmul(
                            pt[:, :256],
                            lhsT=w1t[:, mt, 2 * kp : 2 * kp + 2, 0, :],
                            rhs=xt[:, 2 * kp : 2 * kp + 2, 1, c0 : c0 + 256],
                            start=False, stop=(kp == NK // 2 - 1), perf_mode=DR,
                        )
                    nc.scalar.activation(
                        out=hf[:, c0 : c0 + 256], in_=pt[:, :256],
                        func=AF.Relu, bias=b1t[:, e, mt : mt + 1], scale=SH / S1,
                    )
                nc.gpsimd.tensor_copy(out=hT[:, mt, 0, :], in_=hf)
                nc.vector.tensor_tensor(
                    hT[:, mt, 1, :], hf, hT[:, mt, 0, :], ALU.subtract
                )
            return hT

        def expert_l2(e, hT, w2t, out_pool, tag):
            """L2: oe tile [128, NBT, H2] bf16 = relu(h @ W2 + b2)."""
            oe = out_pool.tile([128, NBT, H2], BF16, tag=tag)
            for bt in range(NBT):
                b0 = bt * 128
                for cb in range(2):
                    c0 = cb * 256
                    pt = ps_l2.tile([128, 512], F32, tag="l2")
                    nc.tensor.matmul(
                        pt[:, :256],
                        lhsT=ones5, rhs=b2t[0:1, e, :, c0 : c0 + 256],
                        start=True, stop=False, perf_mode=DR,
                    )
                    for kt in range(NK):
                        nc.tensor.matmul(
                            pt[:, :256],
                            lhsT=hT[:, kt, 0:1, b0 : b0 + 128].broadcast_to(
                                [128, 2, 128]
                            ),
                            rhs=w2t[:, kt, :, c0 : c0 + 256],
                            start=False, stop=False, perf_mode=DR,
                        )
                    for kp in range(NK // 2):
                        nc.tensor.matmul(
                            pt[:, :256],
                            lhsT=hT[:, 2 * kp : 2 * kp + 2, 1, b0 : b0 + 128],
                            rhs=w2t[:, 2 * kp : 2 * kp + 2, 0, c0 : c0 + 256],
                            start=False, stop=(kp == NK // 2 - 1), perf_mode=DR,
                        )
                    nc.scalar.activation(
                        out=oe[:, bt, c0 : c0 + 256], in_=pt[:, :256],
                        func=AF.Relu, scale=1.0 / S2,
                    )
            return oe

        accs = [None] * 4

        def accumulate(acc_idx, oe, gw, col, first):
            # mul (4x DVE mode) + add (2x) beats fused scalar_tensor_tensor
            # (no fast mode) under the cost model: 442ns vs 594ns per tile.
            acc = accs[acc_idx]
            for bt in range(NBT):
                if first:
                    nc.vector.tensor_scalar_mul(
                        acc[:, bt, :], oe[:, bt, :], gw[:, bt, col : col + 1]
                    )
                else:
                    tmp = p_tmp.tile([128, H2], BF16, tag="tmp", name="tmp")
                    nc.vector.tensor_scalar_mul(
                        tmp, oe[:, bt, :], gw[:, bt, col : col + 1]
                    )
                    nc.vector.tensor_tensor(
                        acc[:, bt, :], acc[:, bt, :], tmp, ALU.add
                    )

        def accumulate2(oe, gw_a, acc_a, col_a, gw_b, acc_b, col_b):
            """Per-bt interleaved double accumulate (shortens the tail)."""
            for bt in range(NBT):
                for gw, ai, col in ((gw_a, acc_a, col_a), (gw_b, acc_b, col_b)):
                    acc = accs[ai]
                    tmp = p_tmp.tile([128, H2], BF16, tag="tmp", name="tmp")
                    nc.vector.tensor_scalar_mul(
                        tmp, oe[:, bt, :], gw[:, bt, col : col + 1]
                    )
                    nc.vector.tensor_tensor(
                        acc[:, bt, :], acc[:, bt, :], tmp, ALU.add
                    )

        def store(acc_idx, y_dram):
            yr = y_dram[:].rearrange("(bt p) o -> bt p o", p=128)
            for bt in range(NBT):
                nc.sync.dma_start(out=yr[bt], in_=accs[acc_idx][:, bt, :])

        # ---- software-pipelined schedule ----
        # Head DMA order: x_shared + shared-gate weights + first W1 slabs
        # first; everything else after.
        xt_sh = load_x(0)
        # W1(0) in quarters with b1 interleaved after the first quarter:
        # q1 unblocks L1 mt0-1, b1 unblocks the first ACT drain.
        w1t = p_w1.tile([128, NM, NK, 2, 128], F8, tag="w1")
        w1src = w1a[0].rearrange("m p k two j -> p m k two j")
        nc.sync.dma_start(out=w1t[:, 0:2], in_=w1src[:, 0:2])
        b1t = p_bias.tile([128, NEXP, NM], F32, tag="b1")
        nc.sync.dma_start(out=b1t, in_=b1a[:])
        for q in range(1, 4):
            nc.sync.dma_start(
                out=w1t[:, 2 * q : 2 * q + 2], in_=w1src[:, 2 * q : 2 * q + 2]
            )
        wsgt = p_wg.tile([128, NK, 2, TOTAL_E], F8, tag="wsg")
        nc.sync.dma_start(out=wsgt, in_=wsga[:])
        bsgt = p_bias.tile([1, TOTAL_E], BF16, tag="bsg")
        nc.sync.dma_start(out=bsgt, in_=bsga[:])

        # shared expert 0 (the shared gate runs after it: its result is
        # not needed until the first combine, and putting it later keeps
        # its weight/bias DMAs off the critical head path)
        w1t_n = load_w1(1)
        w2t0 = load_w2(0)
        hT0 = expert_l1(0, xt_sh, w1t)
        gws = gate(xt_sh, wsgt, bsgt, TOTAL_E, tag="gws")
        # shared expert 1
        w1t = w1t_n
        b2t = p_bias.tile([1, NEXP, 2, H2], F8E5, tag="b2")
        nc.sync.dma_start(out=b2t, in_=b2a[:])
        w2t1 = load_w2(1)
        xt0 = load_x(1)
        bgt = p_bias.tile([1, DOM, GATE_K], BF16, tag="bg")
        nc.sync.dma_start(out=bgt, in_=bga[:])
        wgts = []
        for d in range(DOM):
            wgt = p_wg.tile([128, NK, 2, GATE_K], F8, tag=f"wg{d}")
            nc.sync.dma_start(out=wgt, in_=wga[d])
            wgts.append(wgt)
        hT1 = expert_l1(1, xt_sh, w1t)
        osh0 = expert_l2(0, hT0, w2t0, p_osh, tag="osh0")
        gw0 = gate(xt0, wgts[0], bgt[0:1, 0, :], GATE_K, tag="gw0")

        for i in range(4):
            accs[i] = p_acc.tile(
                [128, NBT, H2], BF16, tag=f"acc{i}", name=f"acc{i}"
            )

        # spec e0 (device 2)
        w1t = load_w1(2)
        w2t2 = load_w2(2)
        hT2 = expert_l1(2, xt0, w1t)
        osh1 = expert_l2(1, hT1, w2t1, p_osh, tag="osh1")
        accumulate(3, osh0, gws, E_SPEC + 0, first=True)
        accumulate(3, osh1, gws, E_SPEC + 1, first=False)
        accumulate(0, osh0, gw0, NES + 0, first=True)
        accumulate(0, osh1, gw0, NES + 1, first=False)

        # spec e1 (device 3)
        w1t = load_w1(3)
        w2t3 = load_w2(3)
        xt1 = load_x(2)
        hT3 = expert_l1(3, xt0, w1t)
        oe = expert_l2(2, hT2, w2t2, p_oe, tag="oe")
        accumulate2(oe, gw0, 0, 0, gws, 3, 0)
        gw1 = gate(xt1, wgts[1], bgt[0:1, 1, :], GATE_K, tag="gw1")

        # spec e2 (device 4)
        w1t = load_w1(4)
        w2t4 = load_w2(4)
        hT4 = expert_l1(4, xt1, w1t)
        oe = expert_l2(3, hT3, w2t3, p_oe, tag="oe")
        accumulate2(oe, gw0, 0, 1, gws, 3, 1)
        store(0, ys[0])
        accumulate(1, osh0, gw1, NES + 0, first=True)
        accumulate(1, osh1, gw1, NES + 1, first=False)

        # spec e3 (device 5)
        w1t = load_w1(5)
        w2t5 = load_w2(5)
        xt2 = load_x(3)
        hT5 = expert_l1(5, xt1, w1t)
        oe = expert_l2(4, hT4, w2t4, p_oe, tag="oe")
        accumulate2(oe, gw1, 1, 0, gws, 3, 2)
        gw2 = gate(xt2, wgts[2], bgt[0:1, 2, :], GATE_K, tag="gw2")

        # spec e4 (device 6)
        w1t = load_w1(6)
        w2t6 = load_w2(6)
        hT6 = expert_l1(6, xt2, w1t)
        oe = expert_l2(5, hT5, w2t5, p_oe, tag="oe")
        accumulate2(oe, gw1, 1, 1, gws, 3, 3)
        store(1, ys[1])
        accumulate(2, osh0, gw2, NES + 0, first=True)
        accumulate(2, osh1, gw2, NES + 1, first=False)

        # spec e5 (device 7)
        w1t = load_w1(7)
        w2t7 = load_w2(7)
        hT7 = expert_l1(7, xt2, w1t)
        oe = expert_l2(6, hT6, w2t6, p_oe, tag="oe")
        accumulate2(oe, gw2, 2, 0, gws, 3, 4)

        # tail: per-bt interleaved final combines + stores
        oe = expert_l2(7, hT7, w2t7, p_oe, tag="oe")
        yr2 = ys[2][:].rearrange("(bt p) o -> bt p o", p=128)
        yr3 = ys[3][:].rearrange("(bt p) o -> bt p o", p=128)
        for bt in range(NBT):
            for gw, ai, col, yr in (
                (gw2, 2, 1, yr2),
                (gws, 3, 5, yr3),
            ):
                acc = accs[ai]
                tmp = p_tmp.tile([128, H2], BF16, tag="tmp", name="tmp")
                nc.vector.tensor_scalar_mul(
                    tmp, oe[:, bt, :], gw[:, bt, col : col + 1]
                )
                nc.vector.tensor_tensor(
                    acc[:, bt, :], acc[:, bt, :], tmp, ALU.add
                )
                nc.sync.dma_start(out=yr[bt], in_=acc[:, bt, :])

    nc.compile()
    return nc


_NC_CACHE = {}


def _get_nc(mm_dt=None):
    key = "fp8dr"
    if key not in _NC_CACHE:
        _NC_CACHE[key] = _build_nc()
    return _NC_CACHE[key]


def _hilo(a, s):
    af = np.asarray(a, np.float32) * np.float32(s)
    hi = af.astype(NPF8)
    lo = (af - hi.astype(np.float32)).astype(NPF8)
    return hi, lo


def _prep_inputs(inputs):
    """Quantize/layout all operands for the device (host-side prep)."""
    f = {k: np.asarray(v, np.float32) for k, v in inputs.items()}

    # x tensors: device order [x_shared, x0, x1, x2]
    x_full = [f["x_shared"], f["x0"], f["x1"], f["x2"]]
    x_per_core = []  # [t][core] -> [128, NK, 2, BL] fp8
    for x in x_full:
        hi, lo = _hilo(x, SX)              # [B, D]
        hi = hi.reshape(B, NK, 128)
        lo = lo.reshape(B, NK, 128)
        cores = []
        for c in range(N_CORES):
            sl = slice(c * BL, (c + 1) * BL)
            xa = np.empty((128, NK, 2, BL), NPF8)
            xa[:, :, 0, :] = hi[sl].transpose(2, 1, 0)
            xa[:, :, 1, :] = lo[sl].transpose(2, 1, 0)
            cores.append(xa)
        x_per_core.append(cores)

    # weights: device expert order [shared0, shared1, spec0..spec5]
    W1 = np.concatenate([f["W1h"], f["W1s"]], axis=0)  # [8, D, H1]
    W2 = np.concatenate([f["W2h"], f["W2s"]], axis=0)  # [8, H1, H2]
    b1 = np.concatenate([f["b1h"], f["b1s"]], axis=0)  # [8, H1]
    b2 = np.concatenate([f["b2h"], f["b2s"]], axis=0)  # [8, H2]

    h1i, l1i = _hilo(W1, SW)
    h1i = h1i.reshape(NEXP, NK, 128, NM, 128)
    l1i = l1i.reshape(NEXP, NK, 128, NM, 128)
    w1a = np.empty((NEXP, NM, 128, NK, 2, 128), NPF8)
    w1a[:, :, :, :, 0, :] = h1i.transpose(0, 3, 2, 1, 4)
    w1a[:, :, :, :, 1, :] = l1i.transpose(0, 3, 2, 1, 4)

    h2i, l2i = _hilo(W2, SW)
    h2i = h2i.reshape(NEXP, NK, 128, H2)
    l2i = l2i.reshape(NEXP, NK, 128, H2)
    w2a = np.empty((NEXP, 128, NK, 2, H2), NPF8)
    w2a[:, :, :, 0, :] = h2i.transpose(0, 2, 1, 3)
    w2a[:, :, :, 1, :] = l2i.transpose(0, 2, 1, 3)

    b1a = np.ascontiguousarray(
        (b1 * SH).reshape(NEXP, NM, 128).transpose(2, 0, 1), dtype=np.float32
    )
    b2s = (b2 * S2).astype(np.float32)
    b2hi = b2s.astype(NPF8E5)
    b2lo = (b2s - b2hi.astype(np.float32)).astype(NPF8E5)
    b2a = np.stack([b2hi, b2lo], axis=1).reshape(1, NEXP, 2, H2)

    hsg, lsg = _hilo(f["Wsg"], SW)  # [D, TOTAL_E]
    wsga = np.empty((128, NK, 2, TOTAL_E), NPF8)
    wsga[:, :, 0, :] = hsg.reshape(NK, 128, TOTAL_E).transpose(1, 0, 2)
    wsga[:, :, 1, :] = lsg.reshape(NK, 128, TOTAL_E).transpose(1, 0, 2)

    hg, lg = _hilo(f["Wg"], SW)  # [DOM, D, GATE_K]
    wga = np.empty((DOM, 128, NK, 2, GATE_K), NPF8)
    wga[:, :, :, 0, :] = hg.reshape(DOM, NK, 128, GATE_K).transpose(0, 2, 1, 3)
    wga[:, :, :, 1, :] = lg.reshape(DOM, NK, 128, GATE_K).transpose(0, 2, 1, 3)

    bsga = (f["bsg"] * S1).reshape(1, TOTAL_E).astype(NPBF)
    bga = (f["bg"] * S1).reshape(1, DOM, GATE_K).astype(NPBF)

    shared = {
        "w1a": w1a, "w2a": w2a, "b1a": b1a, "b2a": b2a,
        "wsga": wsga, "wga": wga, "bsga": bsga, "bga": bga,
    }
    in_maps = []
    for c in range(N_CORES):
        m = dict(shared)
        for t in range(4):
            m[f"xil{t}"] = x_per_core[t][c]
        in_maps.append(m)
    return in_maps


def kernel(**inputs):
    return run_kernel(inputs)


def run_kernel(inputs, mm_dt=None, trace=False):
    nc = _get_nc()
    in_maps = _prep_inputs(inputs)
    res = run_bass_kernel_spmd(nc, in_maps, list(range(N_CORES)), trace=trace)
    outs = []
    for name in ("y0", "y1", "y2", "ysh"):
        outs.append(
            np.concatenate(
                [
                    np.asarray(res.results[c][name]).astype(np.float32)
                    for c in range(N_CORES)
                ],
                axis=0,
            )
        )
    out = tuple(outs)
    if trace:
        return out, res
    return out
